# revision 47
# baseline (speedup 1.0000x reference)
"""Trainium2 Bass kernel for nn_AttentionDecoder (B=4, C=256, H=W=64).

Math (per batch b):
    q  = Wq @ x[b]  + bq          [32, N]   (as qT on device: [32, N] with o on partitions)
    k' = Wk @ xe[b] + bk + pos    [32, N]
    v  = Wv @ xe[b]               [256, N]  (bv folded into epilogue: sum(attn)=1)
    eT = k'^T-chunks: energy^T[m, n] = sum_o k'[o,m] q[o,n]
    pT = exp(eT)                  (no max-subtraction: |energy| < ~30, fp32-exp safe)
    out[c, n] = sum_m v[c, m] pT[m, n]      (PE: lhsT=vT chunk, rhs=pT chunk)
    s[n]      = sum_m pT[m, n]              (PE: lhsT=ones -> replicated rows)
    y = gamma * (out / s + bv) + x

Sharding: 8 cores = (batch, query-half). Each core: 2048 query rows, full m=4096.
"""

import numpy as np
import ml_dtypes
from contextlib import ExitStack

import concourse.bass as bass
import concourse.bacc as bacc
import concourse.tile as tile
import concourse.mybir as mybir
from concourse.bass import ds, ts

B, C, H, W = 4, 256, 64, 64
N = H * W          # 4096
C8 = 32
NH = N // 2        # 2048 query rows per core
NCORES = 8
NG = NH // 512     # 4 n-groups of 512 per core
F32 = mybir.dt.float32
BF16 = mybir.dt.bfloat16
AF = mybir.ActivationFunctionType
OP = mybir.AluOpType

LAST_EXEC_TIME_NS = None
_CACHE = {}


def build_attention(
    ctx,
    tc,
    y,
    ins,
    energy_mode="pack4",
    skip_s=False,
    lag=4,
    ch=2,
    wbufs=2,
    skip_recip=False,
    pv_first=False,
    finish_at=4,
    vt_act_copies=True,
    wvkb_queue="pool",
    late_queue="sp",
    skip_exp=False,
    skip_pv=False,
    ngroups=NG,
    skip_compute=False,
    pe_tail=True,
):
    """y: [128, 2*NH] f32 dram AP.  ins: dict of dram APs (see kernel()).

    energy_mode:
      'pack4'  - 4x row-tiled K=32 matmuls (tile_position), k' strip-split
      'k32'    - plain K=32 matmuls at partitions 0:32 (no tile_position)
      'rep128' - k' replicated on all 4 strips, full K=128 matmul computes
                 4x energy; the 1/4 is folded into exp's scale (free affine)
    """
    nc = tc.nc
    y_v = y.rearrange("p (u n) -> p u n", u=2)
    xqf_d = ins["xqf"].rearrange("p (u n) -> p u n", u=2)
    xqb_d = ins["xqb"].rearrange("p (u n) -> p u n", u=2)
    xeb_d = ins["xeb"].rearrange("p (u m) -> p u m", u=2)

    singles = ctx.enter_context(tc.tile_pool(name="singles", bufs=1))

    # ---- resident SBUF tensors. Input DMAs are spread over three DGE queues
    # (SP, ACT, DVE) so descriptor-gen latency parallelizes, and every chunk
    # carries BOTH u-halves so each consumer unblocks as soon as its chunk
    # lands: k/v production streams behind the xeb quarters.
    # SP queue: xeb first — the whole J-loop gates on k' which needs all of
    # xeb, so its quarters must win the DMA-bandwidth race.
    xeb_sb = singles.tile([128, 2, N], BF16, name="xeb_sb")
    nc.sync.dma_start(xeb_sb[:, :, ds(0, 1024)], xeb_d[:, :, ds(0, 1024)])
    wk_sb = singles.tile([128, 2 * C8], BF16, name="wk_sb")
    nc.sync.dma_start(wk_sb, ins["wk"])
    wq_sb = singles.tile([128, 2, 128], BF16, name="wq_sb")
    nc.sync.dma_start(wq_sb, ins["wq"].rearrange("p (u j) -> p u j", u=2))
    bq_sb = singles.tile([128, 1], F32, name="bq_sb")
    nc.sync.dma_start(bq_sb, ins["bq"])
    # xqb g0 early so qT4 g0 (the only qT4 slice energy g0 needs) is ready
    xqb_sb = singles.tile([128, 2, NH], BF16, name="xqb_sb")
    nc.sync.dma_start(xqb_sb[:, :, ds(0, 512)], xqb_d[:, :, ds(0, 512)])
    for quarter in range(1, 4):
        nc.sync.dma_start(
            xeb_sb[:, :, ds(1024 * quarter, 1024)],
            xeb_d[:, :, ds(1024 * quarter, 1024)],
        )
    consts_sb = singles.tile([128, 3], F32, name="consts_sb")
    nc.sync.dma_start(consts_sb, ins["consts"])
    # wv (vT production) + kbias (k4 add) are needed early; queue selectable
    wvkb_eng = {"pool": nc.gpsimd, "sp": nc.sync, "act": nc.scalar}[wvkb_queue]
    wv_sb = singles.tile([128, 2 * C], BF16, name="wv_sb")
    wvkb_eng.dma_start(wv_sb, ins["wv"])
    kbias_sb = singles.tile([128, 8 * 128], BF16, name="kbias_sb")
    wvkb_eng.dma_start(kbias_sb, ins["kbias"])
    # late-needed tensors go on a queue tail (per-queue FIFO: they can never
    # steal bandwidth from xeb), interleaved by first-need time: xqb g1 feeds
    # emit_q(1) at ~13us, xqf q0 feeds finish(g0) at ~22us, ...
    late_eng = {"pool": nc.gpsimd, "sp": nc.sync, "act": nc.scalar}[late_queue]
    xqf_sb = singles.tile([128, 2, NH], F32, name="xqf_sb")
    late = [("b", 1), ("f", 0), ("b", 2), ("f", 1), ("b", 3), ("f", 2), ("f", 3)]
    for kind, idx in late:
        if kind == "b":
            late_eng.dma_start(
                xqb_sb[:, :, ds(512 * idx, 512)], xqb_d[:, :, ds(512 * idx, 512)]
            )
        else:
            late_eng.dma_start(
                xqf_sb[:, :, ds(512 * idx, 512)], xqf_d[:, :, ds(512 * idx, 512)]
            )

    ones_sb = singles.tile([128, 128], BF16, name="ones_sb")
    nc.vector.memset(ones_sb, 1.0)

    if skip_compute:
        # DMA-only probe: a tiny read of each input tile orders iterations
        probe = singles.tile([128, 8], F32, name="probe")
        for src in (wq_sb, wk_sb, wv_sb, kbias_sb, xqb_sb, xeb_sb, xqf_sb):
            sl = src[:, 0, 0:8] if len(src.shape) == 3 else src[:, 0:8]
            nc.vector.tensor_copy(probe, sl)
        nc.sync.dma_start(y[:, 0:8], probe)
        return

    # warm the Exp ACT table during the production phase (table load ~2.7us)
    act_warm = singles.tile([1, 1], F32, name="act_warm")
    nc.scalar.activation(out=act_warm, in_=bq_sb[0:1, :], func=AF.Exp)

    # qT4: qT replicated at 4 partition strips (strip i holds qT[o, :] at
    # partitions 32i+o) for the row-packed / replicated energyT matmuls.
    qT4_sb = singles.tile([128, NH], BF16, name="qT4_sb")
    # k4: k' distributed over strips: strip i = m in [1024i, 1024(i+1))
    assert energy_mode == "pack4", "col-tiled k production supports pack4 only"
    k4_sb = singles.tile([128, 8 * 128], BF16, name="k4_sb")
    vT_sb = singles.tile([128, 32 * C], BF16, name="vT_sb")

    # ---- productions ----
    # qT4[32i+o, n] = sum_c Wq[o, c] x[c, n] + bq[o]   (wq host-tiled x4)
    qpsum = ctx.enter_context(tc.tile_pool(name="qpsum", bufs=1, space="PSUM"))

    def emit_q(g4, pool):
        psq = pool.tile([128, 512], F32, name="psq")
        for u in range(2):
            nc.tensor.matmul(
                psq,
                lhsT=wq_sb[:, u, :],
                rhs=xqb_sb[:, u, ds(512 * g4, 512)],
                start=(u == 0),
                stop=(u == 1),
            )
        nc.vector.tensor_scalar_add(qT4_sb[:, ds(512 * g4, 512)], psq, bq_sb)

    with tc.tile_pool(name="prodpsum", bufs=2, space="PSUM") as prodpsum:

        # vT[m-chunk mc][mm, co] = sum_c xe[c, 128*mc+mm] Wv[co, c]
        # copies alternate DVE/ACT so neither engine serializes production
        def emit_vt_chunk(pool, mc):
            psv = pool.tile([128, C], F32, name="psv")
            for u in range(2):
                nc.tensor.matmul(
                    psv,
                    lhsT=xeb_sb[:, u, ds(128 * mc, 128)],
                    rhs=wv_sb[:, ds(C * u, C)],
                    start=(u == 0),
                    stop=(u == 1),
                )
            if vt_act_copies and mc % 2 == 1 and mc < 24:
                nc.scalar.copy(vT_sb[:, ds(C * mc, C)], psv)
            else:
                nc.vector.tensor_copy(vT_sb[:, ds(C * mc, C)], psv)

        # k'[o, m] = sum_c Wk[o, c] xe[c, m] + kbias[o, m], produced
        # col-tiled (4 concurrent partition strips) directly in strip layout.
        # Emission follows xeb DMA-arrival order: per quarter q, the k-MMs
        # (strip q) then that quarter's 8 vT chunks.
        emit_q(0, prodpsum)
        psk4 = prodpsum.tile([128, 8 * 128], F32, name="psk4", bufs=1)
        for q in range(4):
            for j in range(2):
                c8 = 2 * q + j
                for u in range(2):
                    nc.tensor.matmul(
                        psk4[ds(C8 * q, C8), ds(512 * j, 512)],
                        lhsT=wk_sb[:, ds(C8 * u, C8)],
                        rhs=xeb_sb[:, u, ds(512 * c8, 512)],
                        start=(u == 0),
                        stop=(u == 1),
                        tile_position=(0, C8 * q),
                    )
            if q == 1:
                # strips 0-1 complete: unblock energy slots 0-15 before the
                # last xeb quarters even land
                nc.vector.tensor_add(
                    k4_sb[0:64, :], psk4[0:64, :], kbias_sb[0:64, :]
                )
            elif q == 3:
                nc.vector.tensor_add(
                    k4_sb[64:128, :], psk4[64:128, :], kbias_sb[64:128, :]
                )
            for mc in range(8 * q, 8 * q + 8):
                emit_vt_chunk(prodpsum, mc)

    # ---- main loop: intra-group pipeline with 2-J-step lag.
    # energyT round J: 4 row-packed K=32 matmuls (strips i=0..3) -> 4 psum
    # banks; exp writes pT slots 4J..4J+3. PV of slots 4(J-2).. runs two
    # J-steps behind so PE has work while ACT drains exp. The s-reduction
    # (DVE tree 32->8 chunks + 8 partition-sum matmuls) and the epilogue of
    # group g overlap group g+1's pipeline fill.
    ppool = ctx.enter_context(tc.tile_pool(name="ppool", bufs=2))
    epool = ctx.enter_context(tc.tile_pool(name="epool", bufs=1, space="PSUM"))
    work = ctx.enter_context(tc.tile_pool(name="work", bufs=wbufs))
    mpsum = ctx.enter_context(tc.tile_pool(name="mpsum", bufs=1, space="PSUM"))
    # PSUM budget: epool 4 + pv0/pv1/s_ps 3 = 7 of 8 banks

    def slot_ij(s):
        # slots 0-15 use k' strips {0,1} (xeb quarters 0-1), 16-31 strips
        # {2,3}: the J-loop can start before the last xeb quarters land
        half = s // 16
        return 2 * half + (s % 2), (s % 16) // 2

    def slot_to_chunk(s):
        i, Jm = slot_ij(s)
        return 8 * i + Jm

    LAG = lag

    def finish(p, last=False):
        """Tree-tail + s-matmuls + normalize + residual + store for group p.

        last=True: the kernel tail — nothing overlaps it, PE is idle, so use
        a shallow DVE tree (st_b only) + 16 s-matmuls instead of the serial
        3-op DVE chain + 4 s-matmuls.
        """
        gp = p["g"]
        s_ps = mpsum.tile([128, 512], F32, name="s_ps")
        if skip_s:
            nc.vector.memset(s_ps, 1.0)
        elif last and pe_tail:
            st_b = work.tile([128, 8, 512], BF16, name="st_b", bufs=1)
            nc.vector.tensor_add(
                st_b, p["pT"][:, 16:24, :], p["pT"][:, 24:32, :]
            )
            for s8 in range(8):
                nc.tensor.matmul(
                    s_ps,
                    lhsT=ones_sb,
                    rhs=p["st_a"][:, s8, :],
                    start=(s8 == 0),
                    stop=False,
                )
            for s8 in range(8):
                nc.tensor.matmul(
                    s_ps,
                    lhsT=ones_sb,
                    rhs=st_b[:, s8, :],
                    start=False,
                    stop=(s8 == 7),
                )
        else:
            st_b = work.tile([128, 8, 512], BF16, name="st_b", bufs=1)
            nc.vector.tensor_add(
                st_b, p["pT"][:, 16:24, :], p["pT"][:, 24:32, :]
            )
            st_c = work.tile([128, 8, 512], BF16, name="st_c", bufs=1)
            nc.vector.tensor_add(st_c, p["st_a"], st_b)
            st3 = work.tile([128, 4, 512], BF16, name="st3", bufs=1)
            nc.vector.tensor_add(st3, st_c[:, 0:4, :], st_c[:, 4:8, :])
            for s8 in range(4):
                nc.tensor.matmul(
                    s_ps,
                    lhsT=ones_sb,
                    rhs=st3[:, s8, :],
                    start=(s8 == 0),
                    stop=(s8 == 3),
                )
        r_rep = work.tile([128, 512], F32, name="r_rep")
        if skip_recip:
            nc.vector.memset(r_rep, 1.0)
        else:
            nc.vector.reciprocal(r_rep, s_ps)
        for u, ou in enumerate([p["o0"], p["o1"]]):
            t = work.tile([128, 512], F32, name="t")
            nc.vector.scalar_tensor_tensor(
                out=t,
                in0=ou,
                scalar=consts_sb[:, 0:1],
                in1=r_rep,
                op0=OP.mult,
                op1=OP.mult,
            )
            yt = work.tile([128, 512], F32, name="yt")
            nc.vector.scalar_tensor_tensor(
                out=yt,
                in0=t,
                scalar=consts_sb[:, u + 1 : u + 2],
                in1=xqf_sb[:, u, ds(512 * gp, 512)],
                op0=OP.add,
                op1=OP.add,
            )
            nc.sync.dma_start(y_v[:, u, ds(512 * gp, 512)], yt)

    # chunk schedule: CH slots per energy/exp chunk; the e-psum tile is CH
    # banks from a bufs=2 ring, so energy(chunk c) only waits on exp(c-2) —
    # one full chunk of slack absorbs ACT/sem latency jitter that a
    # chunk-to-chunk (bufs=1) recycle would put on the critical path.
    CH = ch
    NCHK = 32 // CH
    escale = 0.25 if energy_mode == "rep128" else 1.0

    pending = None
    for g in range(ngroups):
        pT = ppool.tile([128, 32, 512], BF16, name="pT")
        if skip_exp:
            nc.vector.memset(pT[:, 0:1, :], 1.0)
        pv0 = mpsum.tile([128, 512], F32, name="pv0")
        pv1 = mpsum.tile([128, 512], F32, name="pv1")
        st_a = None

        def emit_e(c):
            e_ps = epool.tile(
                [128, CH, 512], F32, name="e_ps", bufs=2 if CH <= 2 else 1
            )
            for j in range(CH):
                s = CH * c + j
                i, Jm = slot_ij(s)
                nc.tensor.matmul(
                    e_ps[:, j, :],
                    lhsT=k4_sb[ds(C8 * i, C8), ds(128 * Jm, 128)],
                    rhs=qT4_sb[ds(C8 * i, C8), ds(512 * g, 512)],
                    start=True,
                    stop=True,
                    tile_position=(C8 * i, 0),
                )
            if not skip_exp:
                nc.scalar.activation(
                    out=pT[:, ds(CH * c, CH), :],
                    in_=e_ps,
                    func=AF.Exp,
                    scale=escale,
                )

        def emit_pv(c):
            for j in range(CH):
                s = CH * (c - LAG) + j
                mc = slot_to_chunk(s)
                st = s == 0
                sp = s == 31
                rhs = pT[:, 0, :] if skip_exp else pT[:, s, :]
                nc.tensor.matmul(
                    pv0,
                    lhsT=vT_sb[:, ds(C * mc, 128)],
                    rhs=rhs,
                    start=st,
                    stop=sp,
                )
                nc.tensor.matmul(
                    pv1,
                    lhsT=vT_sb[:, ds(C * mc + 128, 128)],
                    rhs=rhs,
                    start=st,
                    stop=sp,
                )

        for c in range(NCHK + LAG):
            if pv_first:
                if c >= LAG and not skip_pv:
                    emit_pv(c)
                if c < NCHK:
                    emit_e(c)
            else:
                if c < NCHK:
                    emit_e(c)
                if c >= LAG and not skip_pv:
                    emit_pv(c)
            if c == 1 and g + 1 < NG:
                # produce the NEXT group's qT4 slice here: its xqb chunk
                # lands mid-flight, so doing it up front would head-of-line
                # block the in-order PE queue
                emit_q(g + 1, qpsum)
            if c == 16 // CH and not skip_s:
                # first half of the s slot-tree: slots 0..15 are ready
                st_a = work.tile([128, 8, 512], BF16, name="st_a")
                nc.vector.tensor_add(st_a, pT[:, 0:8, :], pT[:, 8:16, :])
            if c == finish_at and pending is not None:
                finish(pending)
                pending = None
        # evacuate PV psum to SBUF right away so the psum banks free for the
        # next group's PV; the tree-tail + s-matmuls + normalize/epilogue are
        # deferred into the next group's J-loop (see finish())
        o0 = work.tile([128, 512], F32, name="o0")
        o1 = work.tile([128, 512], F32, name="o1")
        if skip_pv:
            nc.vector.memset(o0, 1.0)
            nc.vector.memset(o1, 1.0)
        else:
            nc.vector.tensor_copy(o0, pv0)
            nc.vector.tensor_copy(o1, pv1)
        pending = dict(g=g, pT=pT, o0=o0, o1=o1, st_a=st_a)
    finish(pending, last=True)


INPUT_SPECS = [
    ("xqf", [128, 2 * NH], F32),
    ("xqb", [128, 2 * NH], BF16),
    ("xeb", [128, 2 * N], BF16),
    ("wq", [128, 2 * 128], BF16),
    ("wk", [128, 2 * C8], BF16),
    ("wv", [128, 2 * C], BF16),
    ("kbias", [128, 8 * 128], BF16),
    ("bq", [128, 1], F32),
    ("consts", [128, 3], F32),
]


def _get_program(loop_iters=None, unroll=1, **opts):
    """loop_iters=None: plain program. loop_iters=k: whole kernel wrapped in a
    device-side For_i loop (for HW timing: slope between two loop counts).
    unroll=k (with loop_iters=None): k sequential copies of the body, for
    sim-measuring the steady-state marginal iteration cost."""
    key = ("nc", loop_iters, unroll, tuple(sorted(opts.items())))
    if key not in _CACHE:
        nc = bacc.Bacc("TRN2", debug=False, num_devices=NCORES)
        with tile.TileContext(nc) as tc:
            with ExitStack() as ctx:
                ins = {
                    name: nc.dram_tensor(name, shape, dt, kind="ExternalInput").ap()
                    for name, shape, dt in INPUT_SPECS
                }
                y = nc.dram_tensor("y", [128, 2 * NH], F32, kind="ExternalOutput").ap()
                if loop_iters is None:
                    for _u in range(unroll):
                        with ExitStack() as inner:
                            build_attention(inner, tc, y, ins, **opts)
                else:
                    # hint_engines: body >256 insts/engine, so the back-edge
                    # would otherwise pay an IRAM refetch (~3-4us) per iter
                    with tc.For_i(
                        0,
                        loop_iters,
                        1,
                        hint_engines=(
                            mybir.EngineType.PE,
                            mybir.EngineType.Activation,
                            mybir.EngineType.DVE,
                            mybir.EngineType.SP,
                            mybir.EngineType.Pool,
                        ),
                    ):
                        for _u in range(unroll):
                            with ExitStack() as inner:
                                build_attention(inner, tc, y, ins, **opts)
        nc.compile()
        _CACHE[key] = nc
    return _CACHE[key]


class _Runner:
    """Executes the compiled Bass program on 8 cores via PJRT/axon.

    Mirrors bass2jax.run_bass_via_pjrt's multi-core path, but keeps the
    jitted callable so repeated executions don't re-lower, and supports
    chaining `iters` NEFF executions inside one program (each iteration's
    outputs feed the next iteration's output buffers, creating a data
    dependency) so per-execution device time can be measured without
    host dispatch overhead.
    """

    def __init__(self, nc):
        import jax
        from jax.experimental.shard_map import shard_map
        from jax.sharding import Mesh, PartitionSpec
        from concourse import bass2jax

        bass2jax.install_neuronx_cc_hook()
        self.nc = nc
        self.jax = jax
        in_names, out_names, out_avals, zero_outs = [], [], [], []
        partition_name = (
            nc.partition_id_tensor.name if nc.partition_id_tensor else None
        )
        for alloc in nc.m.functions[0].allocations:
            if not isinstance(alloc, mybir.MemoryLocationSet):
                continue
            name = alloc.memorylocations[0].name
            if alloc.kind == "ExternalInput":
                if name != partition_name:
                    in_names.append(name)
            elif alloc.kind == "ExternalOutput":
                out_names.append(name)
                shape = tuple(alloc.tensor_shape)
                dtype = mybir.dt.np(alloc.dtype)
                out_avals.append(jax.core.ShapedArray(shape, dtype))
                zero_outs.append(np.zeros(shape, dtype))
        self.n_params = len(in_names)
        self.n_outs = len(out_avals)
        self.out_names = out_names
        self.out_avals = out_avals
        self.zero_outs = zero_outs
        all_in_names = list(in_names) + list(out_names)
        if partition_name is not None:
            all_in_names.append(partition_name)
        self.in_names = in_names
        self.partition_name = partition_name

        devices = jax.devices()[:NCORES]
        assert len(devices) == NCORES
        mesh = Mesh(np.asarray(devices), ("core",))
        donate = tuple(range(self.n_params, self.n_params + self.n_outs))
        out_avals_t = tuple(out_avals)
        all_in_names_t = tuple(all_in_names)
        out_names_t = tuple(out_names)

        self.mesh = mesh
        self.pspec = PartitionSpec("core")

        def make(donated):
            def _body(*args):
                operands = list(args)
                if partition_name is not None:
                    operands.append(bass2jax.partition_id_tensor())
                outs = bass2jax._bass_exec_p.bind(
                    *operands,
                    out_avals=out_avals_t,
                    in_names=all_in_names_t,
                    out_names=out_names_t,
                    lowering_input_output_aliases=(),
                    sim_require_finite=True,
                    sim_require_nnan=True,
                    nc=nc,
                )
                return tuple(outs)

            in_specs = (PartitionSpec("core"),) * (self.n_params + self.n_outs)
            out_specs = (PartitionSpec("core"),) * self.n_outs
            return jax.jit(
                shard_map(
                    _body,
                    mesh=mesh,
                    in_specs=in_specs,
                    out_specs=out_specs,
                    check_rep=False,
                ),
                donate_argnums=donate if donated else (),
                keep_unused=True,
            )

        self._make = make
        self._fns = {}

    def _fn(self, donated):
        if donated not in self._fns:
            self._fns[donated] = self._make(donated)
        return self._fns[donated]

    def _concat_args(self, in_maps):
        concat_in = [
            np.concatenate([np.asarray(m[name]) for m in in_maps], axis=0)
            for name in self.in_names
        ]
        concat_zeros = [
            np.zeros((NCORES * z.shape[0], *z.shape[1:]), z.dtype)
            for z in self.zero_outs
        ]
        return concat_in + concat_zeros

    def device_args(self, in_maps):
        """Pre-place sharded args on the 8 devices (for re-execution timing)."""
        jax = self.jax
        from jax.sharding import NamedSharding

        sharding = NamedSharding(self.mesh, self.pspec)
        return [jax.device_put(a, sharding) for a in self._concat_args(in_maps)]

    def execute(self, dev_args):
        """Run on pre-placed device args without donation; returns jax arrays."""
        return self._fn(False)(*dev_args)

    def run(self, in_maps):
        out_arrs = self._fn(True)(*self._concat_args(in_maps))
        out_arrs = [np.asarray(a) for a in out_arrs]
        return [
            {
                name: out_arrs[i].reshape(NCORES, *self.out_avals[i].shape)[c]
                for i, name in enumerate(self.out_names)
            }
            for c in range(NCORES)
        ]


def get_runner():
    if "runner" not in _CACHE:
        _CACHE["runner"] = _Runner(_get_program())
    return _CACHE["runner"]


def get_loop_runner(loop_iters, **opts):
    key = ("runner", loop_iters, tuple(sorted(opts.items())))
    if key not in _CACHE:
        _CACHE[key] = _Runner(_get_program(loop_iters, **opts))
    return _CACHE[key]


def measure_hw_ns(in_maps, k_lo=1, k_hi=129, reps=6, **opts):
    """Per-iteration device time via two For_i loop-count variants."""
    import time as _time
    import jax as _jax

    def bench(runner):
        dev = runner.device_args(in_maps)
        for _ in range(2):
            _jax.block_until_ready(runner.execute(dev))
        best = float("inf")
        for _ in range(reps):
            t0 = _time.perf_counter()
            _jax.block_until_ready(runner.execute(dev))
            best = min(best, _time.perf_counter() - t0)
        return best

    t_lo = bench(get_loop_runner(k_lo, **opts))
    t_hi = bench(get_loop_runner(k_hi, **opts))
    return (t_hi - t_lo) / (k_hi - k_lo) * 1e9, t_lo, t_hi


def get_trivial_runner():
    """Near-empty NEFF (one tiny DMA in->out) to measure dispatch overhead."""
    if "trivial" not in _CACHE:
        nc = bacc.Bacc("TRN2", debug=False, num_devices=NCORES)
        with tile.TileContext(nc) as tc:
            with ExitStack() as ctx:
                tin = nc.dram_tensor("tin", [128, 8], F32, kind="ExternalInput").ap()
                tout = nc.dram_tensor(
                    "tout", [128, 8], F32, kind="ExternalOutput"
                ).ap()
                pool = ctx.enter_context(tc.tile_pool(name="tpool", bufs=1))
                tt = pool.tile([128, 8], F32, name="tt")
                nc.sync.dma_start(tt, tin)
                nc.sync.dma_start(tout, tt)
        nc.compile()
        _CACHE["trivial"] = _Runner(nc)
    return _CACHE["trivial"]


def _to2(a):
    """[256, X] -> [128, 2X] with out[p, u*X + j] = a[128u + p, j]."""
    x = np.asarray(a)
    return np.ascontiguousarray(
        x.reshape(2, 128, x.shape[1]).transpose(1, 0, 2).reshape(128, -1)
    )


def _bf(a):
    return np.ascontiguousarray(np.asarray(a, dtype=ml_dtypes.bfloat16))


def _f32(a):
    return np.ascontiguousarray(np.asarray(a, dtype=np.float32))


def kernel(x, x_encoder, Wq, bq, Wk, bk, Wv, bv, h_pos, w_pos, gamma):
    global LAST_EXEC_TIME_NS
    in_maps = make_in_maps(
        x, x_encoder, Wq, bq, Wk, bk, Wv, bv, h_pos, w_pos, gamma
    )
    runner = get_runner()
    results = runner.run(in_maps)

    out = np.empty((B, C, N), np.float32)
    for core in range(NCORES):
        b, half = divmod(core, 2)
        yc = results[core]["y"]  # [128, 2*NH]
        out[b][:, half * NH : (half + 1) * NH] = (
            yc.reshape(128, 2, NH).transpose(1, 0, 2).reshape(C, NH)
        )
    return out.reshape(B, C, H, W)


def make_in_maps(x, x_encoder, Wq, bq, Wk, bk, Wv, bv, h_pos, w_pos, gamma):
    """Host-side input prep shared by kernel() and timing harnesses."""
    x = _f32(x)
    x_encoder = _f32(x_encoder)
    Wq, bq, Wk, bk, Wv, bv = map(_f32, (Wq, bq, Wk, bk, Wv, bv))
    h_pos, w_pos, gamma = map(_f32, (h_pos, w_pos, gamma))
    xf = x.reshape(B, C, N)
    xe = x_encoder.reshape(B, C, N)
    pos = (h_pos + w_pos).reshape(C8, N)
    kb = bk[:, None] + pos  # [32, 4096]
    # strip layout: kbias4[32i+o, j] = kb[o, 1024i + j]
    kbias = _bf(kb.reshape(C8, 4, 8 * 128).transpose(1, 0, 2).reshape(128, 8 * 128))
    wqT = Wq.T  # [256, 32]
    wq_h = _bf(
        np.concatenate(
            [np.tile(wqT[128 * u : 128 * (u + 1)], (1, 4)) for u in range(2)],
            axis=1,
        )
    )  # [128, 256]: wq_h[p, 128u + 32i + o] = Wq[o, 128u + p]
    wk_h = _bf(_to2(Wk.T))
    wv_h = _bf(_to2(Wv.T))
    bq_h = _f32(np.tile(bq, 4)[:, None])  # [128, 1]
    g = float(gamma.reshape(-1)[0])
    consts = np.empty((128, 3), np.float32)
    consts[:, 0] = g
    consts[:, 1] = g * bv[0:128]
    consts[:, 2] = g * bv[128:256]
    in_maps = []
    for core in range(NCORES):
        b, half = divmod(core, 2)
        xq = _to2(xf[b][:, half * NH : (half + 1) * NH])
        in_maps.append(
            {
                "xqf": _f32(xq),
                "xqb": _bf(xq),
                "xeb": _bf(_to2(xe[b])),
                "wq": wq_h,
                "wk": wk_h,
                "wv": wv_h,
                "kbias": kbias,
                "bq": bq_h,
                "consts": consts,
            }
        )
    return in_maps


if __name__ == "__main__":
    import reference

    inputs = {k: np.asarray(v) for k, v in reference.setup_inputs().items()}
    got = kernel(**inputs)
    print("kernel ran; output shape", got.shape, "exec_ns", LAST_EXEC_TIME_NS)

